# revision 4
# baseline (speedup 1.0000x reference)
"""BalancedErrorRateLoss Trainium2 kernel.

Computes: err[i] = |1 - input_[i, target[i]]|; per-group means of err over
`group` (8 groups); loss = |0.5 - mean(group_means)|.

Strategy (data-parallel over N across 8 NeuronCores):
  - Each core gets N/8 = 524288 rows, laid out partition-major as
    [128 partitions, 4096 rows/partition].
  - Gather input_[i, target[i]] on-chip with a two-stage 4-way predicated
    select (16 -> 4 -> 1) driven by bit-plane masks of `target` prepared on
    host (pure index reformatting).
  - err = Abs(1 - sel) on the Scalar engine.
  - Per-group sums and counts via fused scalar_tensor_tensor passes
    (mask==g, multiply, per-partition accumulate).
  - Partition-axis reduction via a single [128,16]x[128,1] matmul into PSUM.
  - Host combines the 8 per-core [sums|counts] partials into the scalar.
"""

import sys
import os

for _p in ("/opt/trn_rl_repo",):
    if os.path.isdir(_p) and _p not in sys.path:
        sys.path.append(_p)

import numpy as np

N, C, G = 4_194_304, 16, 8
CORES = 8
ROWS = N // CORES          # 524288 rows per core
P = 128                    # partitions
RPPT = ROWS // P           # 4096 rows per partition (total)
NT = 8                     # tiles per core
RPP = RPPT // NT           # 512 rows per partition per tile
XF = RPP * C               # 8192 x-elements per partition per tile

_CACHE = {}


def _build_nc():
    import concourse.bacc as bacc
    import concourse.tile as tile
    from concourse import mybir
    from contextlib import ExitStack

    f32 = mybir.dt.float32
    nc = bacc.Bacc("TRN2", target_bir_lowering=False, debug=False,
                   num_devices=CORES)

    x = nc.dram_tensor("x", [P, RPPT * C], f32, kind="ExternalInput").ap()
    u8 = mybir.dt.uint8
    planes = {}
    for nm in ("m1", "m2", "m3", "M1", "M2", "M3"):
        planes[nm] = nc.dram_tensor(nm, [P, RPPT], u8,
                                    kind="ExternalInput").ap()
    planes["g"] = nc.dram_tensor("g", [P, RPPT], f32,
                                 kind="ExternalInput").ap()
    part = nc.dram_tensor("part", [16, 1], f32, kind="ExternalOutput").ap()

    with tile.TileContext(nc) as tc, ExitStack() as ctx:
        xp = ctx.enter_context(tc.tile_pool(name="xp", bufs=2))
        mp = ctx.enter_context(tc.tile_pool(name="mp", bufs=2))
        sp = ctx.enter_context(tc.tile_pool(name="sp", bufs=2))
        bigp = ctx.enter_context(tc.tile_pool(name="bigp", bufs=1))
        psp = ctx.enter_context(tc.tile_pool(name="psp", bufs=1, space="PSUM"))

        g_all = bigp.tile([P, RPPT], f32)
        nc.sync.dma_start(g_all[:], planes["g"][:])
        err_all = bigp.tile([P, RPPT], f32)

        for ti in range(NT):
            xt = xp.tile([P, XF], f32, tag="x")
            nc.sync.dma_start(xt[:], x[:, ti * XF:(ti + 1) * XF])
            mts = {}
            for nm in ("m1", "m2", "m3", "M1", "M2", "M3"):
                t = mp.tile([P, RPP], u8, tag=nm)
                nc.sync.dma_start(t[:], planes[nm][:, ti * RPP:(ti + 1) * RPP])
                mts[nm] = t

            # stage 1: 16 -> 4 by low 2 bits of target
            x4 = xt[:].rearrange("p (j u v) -> p j u v", u=4, v=4)
            s4 = sp.tile([P, RPP * 4], f32, tag="s4")
            s4v = s4[:].rearrange("p (j u) -> p j u", u=4)
            nc.vector.tensor_copy(s4v, x4[:, :, :, 0])
            for i, nm in enumerate(("m1", "m2", "m3")):
                mb = mts[nm][:].rearrange("p (j o) -> p j o", o=1)
                mb = mb.broadcast_to((P, RPP, 4))
                nc.vector.copy_predicated(s4v, mb, x4[:, :, :, i + 1])

            # stage 2: 4 -> 1 by high 2 bits of target
            s4u = s4[:].rearrange("p (j u) -> p j u", u=4)
            sel = sp.tile([P, RPP], f32, tag="sel")
            nc.vector.tensor_copy(sel[:], s4u[:, :, 0])
            for i, nm in enumerate(("M1", "M2", "M3")):
                nc.vector.copy_predicated(sel[:], mts[nm][:], s4u[:, :, i + 1])

            # err = |1 - sel| on the scalar engine
            nc.scalar.activation(err_all[:, ti * RPP:(ti + 1) * RPP], sel[:],
                                 mybir.ActivationFunctionType.Abs,
                                 bias=1.0, scale=-1.0)

        # group sums and counts: fused mask+multiply+accumulate passes
        acc = bigp.tile([P, 16], f32)
        scr = bigp.tile([P, RPPT], f32)
        for gv in range(G):
            nc.vector.scalar_tensor_tensor(
                scr[:], g_all[:], float(gv), err_all[:],
                mybir.AluOpType.is_equal, mybir.AluOpType.mult,
                accum_out=acc[:, gv:gv + 1])
        for gv in range(G):
            nc.vector.scalar_tensor_tensor(
                scr[:], g_all[:], float(gv), g_all[:],
                mybir.AluOpType.is_equal, mybir.AluOpType.bypass,
                accum_out=acc[:, 8 + gv:8 + gv + 1])

        # partition-axis reduction: ones^T accumulate via matmul into PSUM
        ones = bigp.tile([P, 1], f32)
        nc.gpsimd.memset(ones[:], 1.0)
        ps = psp.tile([16, 1], f32)
        nc.tensor.matmul(ps[:], lhsT=acc[:], rhs=ones[:],
                         start=True, stop=True)
        res_sb = bigp.tile([16, 1], f32)
        nc.vector.tensor_copy(res_sb[:], ps[:])
        nc.sync.dma_start(part[:], res_sb[:])

    nc.compile()
    return nc


def _get_nc():
    if "nc" not in _CACHE:
        _CACHE["nc"] = _build_nc()
    return _CACHE["nc"]


def make_in_maps(input_, target, group):
    x = np.ascontiguousarray(np.asarray(input_, dtype=np.float32))
    t = np.asarray(target).astype(np.int32)
    g = np.asarray(group).astype(np.float32)
    in_maps = []
    for c in range(CORES):
        sl = slice(c * ROWS, (c + 1) * ROWS)
        tl = t[sl].reshape(P, RPPT)
        lo = tl & 3
        hi = tl >> 2
        in_maps.append({
            "x": x[sl].reshape(P, RPPT * C),
            "m1": (lo == 1).astype(np.uint8),
            "m2": (lo == 2).astype(np.uint8),
            "m3": (lo == 3).astype(np.uint8),
            "M1": (hi == 1).astype(np.uint8),
            "M2": (hi == 2).astype(np.uint8),
            "M3": (hi == 3).astype(np.uint8),
            "g": g[sl].reshape(P, RPPT),
        })
    return in_maps


def finish(parts):
    """parts: [CORES, 16] array of per-core [sums(8) | counts(8)]."""
    parts = np.asarray(parts, dtype=np.float32)
    sums = parts[:, :8].sum(axis=0, dtype=np.float32)
    counts = parts[:, 8:].sum(axis=0, dtype=np.float32)
    means = np.where(counts > 0, sums / np.maximum(counts, 1.0),
                     np.float32(0.0)).astype(np.float32)
    return np.float32(abs(np.float32(0.5) - means.mean(dtype=np.float32)))


def kernel(input_, target, group):
    from concourse import bass_utils

    nc = _get_nc()
    in_maps = make_in_maps(input_, target, group)
    res = bass_utils.run_bass_kernel_spmd(nc, in_maps,
                                          core_ids=list(range(CORES)))
    parts = np.stack([res.results[c]["part"].reshape(16) for c in range(CORES)])
    return finish(parts)


if __name__ == "__main__":
    rng = np.random.default_rng(0)
    x = rng.normal(size=(N, C)).astype(np.float32)
    t = rng.integers(0, C, size=N).astype(np.int32)
    g = rng.integers(0, G, size=N).astype(np.int32)
    out = kernel(input_=x, target=t, group=g)
    err = np.abs(1.0 - x[np.arange(N), t])
    sums = np.bincount(g, weights=err, minlength=G)
    counts = np.bincount(g, minlength=G)
    means = np.where(counts > 0, sums / np.maximum(counts, 1), 0.0)
    exp = abs(0.5 - means.mean())
    print("kernel:", out, "expected:", exp, "rel:", abs(out - exp) / abs(exp))


# revision 6
# speedup vs baseline: 1.4408x; 1.4408x over previous
"""BalancedErrorRateLoss Trainium2 kernel.

Computes: err[i] = |1 - input_[i, target[i]]|; per-group means of err over
`group` (8 groups); loss = |0.5 - mean(group_means)|.

Strategy (data-parallel over N across 8 NeuronCores):
  - Each core gets N/8 = 524288 rows, laid out partition-major as
    [128 partitions, 4096 rows/partition], in bf16, with the 16 channels
    stored lane-major per tile ([tile, channel, row]) so every DVE read is
    contiguous (unlocks 2x/4x DVE perf modes).
  - Gather input_[i, target[i]] on-chip with a two-stage 4-way predicated
    select (16 -> 4 -> 1) driven by uint16 bit-plane masks of `target`
    prepared on host (pure index reformatting).
  - err = Abs(1 - sel) on the Scalar engine.
  - Per-group sums via fused scalar_tensor_tensor (mask==g * err, accum);
    counts via single-src tensor_scalar (mask==g, accum).
  - Partition-axis reduction via one [128,16]x[128,1] matmul into PSUM.
  - Host combines the 8 per-core [sums|counts] partials into the scalar.
"""

import sys
import os

for _p in ("/opt/trn_rl_repo",):
    if os.path.isdir(_p) and _p not in sys.path:
        sys.path.append(_p)

import numpy as np
import ml_dtypes

BF16 = np.dtype(ml_dtypes.bfloat16)

N, C, G = 4_194_304, 16, 8
CORES = 8
ROWS = N // CORES          # 524288 rows per core
P = 128                    # partitions
RPPT = ROWS // P           # 4096 rows per partition (total)
NT = 8                     # tiles per core
RPP = RPPT // NT           # 512 rows per partition per tile
XF = RPP * C               # 8192 x-elements per partition per tile

_CACHE = {}


def _build_nc():
    import concourse.bacc as bacc
    import concourse.tile as tile
    from concourse import mybir
    from contextlib import ExitStack

    f32 = mybir.dt.float32
    bf16 = mybir.dt.bfloat16
    u16 = mybir.dt.uint16
    nc = bacc.Bacc("TRN2", target_bir_lowering=False, debug=False,
                   num_devices=CORES)

    # x: per-tile lane-major bf16: column = ti*XF + k*RPP + j
    x = nc.dram_tensor("x", [P, RPPT * C], bf16, kind="ExternalInput").ap()
    # aux: per-tile packed planes: 6 masks (u16) + g (bf16 bits in u16)
    aux = nc.dram_tensor("aux", [P, NT * 7 * RPP], u16,
                         kind="ExternalInput").ap()
    part = nc.dram_tensor("part", [16, 1], f32, kind="ExternalOutput").ap()

    with tile.TileContext(nc) as tc, ExitStack() as ctx:
        xp = ctx.enter_context(tc.tile_pool(name="xp", bufs=3))
        ap_ = ctx.enter_context(tc.tile_pool(name="ap", bufs=3))
        sp = ctx.enter_context(tc.tile_pool(name="sp", bufs=2))
        bigp = ctx.enter_context(tc.tile_pool(name="bigp", bufs=1))
        psp = ctx.enter_context(tc.tile_pool(name="psp", bufs=1, space="PSUM"))

        g_all = bigp.tile([P, RPPT], bf16)
        err_all = bigp.tile([P, RPPT], bf16)

        for ti in range(NT):
            xt = xp.tile([P, XF], bf16, tag="x")
            nc.sync.dma_start(xt[:], x[:, ti * XF:(ti + 1) * XF])
            auxt = ap_.tile([P, 7 * RPP], u16, tag="aux")
            nc.sync.dma_start(auxt[:],
                              aux[:, ti * 7 * RPP:(ti + 1) * 7 * RPP])

            masks = [auxt[:, i * RPP:(i + 1) * RPP] for i in range(6)]
            gt = auxt[:, 6 * RPP:7 * RPP].bitcast(bf16)

            # stage 1: 16 -> 4 by low 2 bits of target (v = t & 3)
            x3 = xt[:].rearrange("p (u v j) -> p u v j", u=4, v=4)
            s4 = sp.tile([P, RPP * 4], bf16, tag="s4")
            s4v = s4[:].rearrange("p (u j) -> p u j", u=4)
            nc.vector.tensor_copy(s4v, x3[:, :, 0, :])
            for i in range(3):
                mb = masks[i].rearrange("p (o j) -> p o j", o=1)
                mb = mb.broadcast_to((P, 4, RPP))
                nc.vector.copy_predicated(s4v, mb, x3[:, :, i + 1, :])

            # stage 2: 4 -> 1 by high 2 bits of target (u = t >> 2)
            s4u = s4[:].rearrange("p (u j) -> p u j", u=4)
            sel = sp.tile([P, RPP], bf16, tag="sel")
            nc.vector.tensor_copy(sel[:], s4u[:, 0, :])
            for i in range(3):
                nc.vector.copy_predicated(sel[:], masks[3 + i],
                                          s4u[:, i + 1, :])

            # err = |1 - sel| on the scalar engine; stash g plane
            nc.scalar.activation(err_all[:, ti * RPP:(ti + 1) * RPP], sel[:],
                                 mybir.ActivationFunctionType.Abs,
                                 bias=1.0, scale=-1.0)
            nc.vector.tensor_copy(g_all[:, ti * RPP:(ti + 1) * RPP], gt)

        # group sums (stt, 2-input) and counts (tensor_scalar, 1-input)
        acc = bigp.tile([P, 16], f32)
        scr = bigp.tile([P, RPPT], bf16)
        for gv in range(G):
            nc.vector.scalar_tensor_tensor(
                scr[:], g_all[:], float(gv), err_all[:],
                mybir.AluOpType.is_equal, mybir.AluOpType.mult,
                accum_out=acc[:, gv:gv + 1])
        for gv in range(G):
            nc.vector.tensor_scalar(
                scr[:], g_all[:], float(gv), None,
                mybir.AluOpType.is_equal, mybir.AluOpType.add,
                accum_out=acc[:, 8 + gv:8 + gv + 1])

        # partition-axis reduction: ones^T accumulate via matmul into PSUM
        ones = bigp.tile([P, 1], f32)
        nc.gpsimd.memset(ones[:], 1.0)
        ps = psp.tile([16, 1], f32)
        nc.tensor.matmul(ps[:], lhsT=acc[:], rhs=ones[:],
                         start=True, stop=True)
        res_sb = bigp.tile([16, 1], f32)
        nc.vector.tensor_copy(res_sb[:], ps[:])
        nc.sync.dma_start(part[:], res_sb[:])

    nc.compile()
    return nc


def _get_nc():
    if "nc" not in _CACHE:
        _CACHE["nc"] = _build_nc()
    return _CACHE["nc"]


def _to_bf16_bits(x_f32):
    """f32 -> bf16 (round-to-nearest-even) as uint16 bit patterns."""
    u = x_f32.view(np.uint32)
    rounded = (u + 0x7FFF + ((u >> 16) & 1)) >> 16
    return rounded.astype(np.uint16)


def make_in_maps(input_, target, group):
    x = np.ascontiguousarray(np.asarray(input_, dtype=np.float32))
    t = np.asarray(target).astype(np.int32)
    g = np.asarray(group).astype(np.int32)
    in_maps = []
    for c in range(CORES):
        sl = slice(c * ROWS, (c + 1) * ROWS)
        # x: [128, NT, RPP, 16] -> lane-major [128, NT, 16, RPP], bf16 bits
        xc = x[sl].reshape(P, NT, RPP, C).transpose(0, 1, 3, 2)
        xb = _to_bf16_bits(np.ascontiguousarray(xc)).view(BF16)
        tl = t[sl].reshape(P, NT, RPP)
        lo = tl & 3
        hi = tl >> 2
        gb = _to_bf16_bits(g[sl].reshape(P, NT, RPP).astype(np.float32))
        auxc = np.stack([
            (lo == 1).astype(np.uint16), (lo == 2).astype(np.uint16),
            (lo == 3).astype(np.uint16), (hi == 1).astype(np.uint16),
            (hi == 2).astype(np.uint16), (hi == 3).astype(np.uint16),
            gb,
        ], axis=2)  # [P, NT, 7, RPP]
        in_maps.append({
            "x": xb.reshape(P, RPPT * C),
            "aux": np.ascontiguousarray(auxc).reshape(P, NT * 7 * RPP),
        })
    return in_maps


def finish(parts):
    """parts: [CORES, 16] array of per-core [sums(8) | counts(8)]."""
    parts = np.asarray(parts, dtype=np.float32)
    sums = parts[:, :8].sum(axis=0, dtype=np.float32)
    counts = parts[:, 8:].sum(axis=0, dtype=np.float32)
    means = np.where(counts > 0, sums / np.maximum(counts, 1.0),
                     np.float32(0.0)).astype(np.float32)
    return np.float32(abs(np.float32(0.5) - means.mean(dtype=np.float32)))


def kernel(input_, target, group):
    from concourse import bass_utils

    nc = _get_nc()
    in_maps = make_in_maps(input_, target, group)
    res = bass_utils.run_bass_kernel_spmd(nc, in_maps,
                                          core_ids=list(range(CORES)))
    parts = np.stack([res.results[c]["part"].reshape(16) for c in range(CORES)])
    return finish(parts)


if __name__ == "__main__":
    rng = np.random.default_rng(0)
    x = rng.normal(size=(N, C)).astype(np.float32)
    t = rng.integers(0, C, size=N).astype(np.int32)
    g = rng.integers(0, G, size=N).astype(np.int32)
    out = kernel(input_=x, target=t, group=g)
    err = np.abs(1.0 - x[np.arange(N), t])
    sums = np.bincount(g, weights=err, minlength=G)
    counts = np.bincount(g, minlength=G)
    means = np.where(counts > 0, sums / np.maximum(counts, 1), 0.0)
    exp = abs(0.5 - means.mean())
    print("kernel:", out, "expected:", exp, "rel:", abs(out - exp) / abs(exp))
